# revision 1
# baseline (speedup 1.0000x reference)
"""Multi-head causal attention (B=2, S=2048, E=1024, H=16, D=64) on 8 TRN2
NeuronCores.

Sharding: 4-way tensor-parallel over heads x 2-way data-parallel over batch.
Core c handles batch b = c//4 and head group g = c%4 (heads 4g..4g+3).

v2 device program (software-pipelined emission):
  - Q/K projections in fp8e4m3 with DoubleRow matmuls (2 e-planes per
    instruction, 0.5 cycles/row); V / out projections stay bf16 for
    accuracy. All PSUM accumulation fp32.
  - QT/KT computed in [d, q] layout per q chunk; V in [s, d] with a ones
    column per head (V_aug) so softmax denominators fall out of the PV
    matmul for free.
  - Scores TRANSPOSED: ST[k, q] via lhsT=KT, rhs=QT; the two heads of a
    pair run on PE row groups 0-63 / 64-127 concurrently, writing one
    2-bank PSUM tile [128, 2, 512] so a SINGLE wide exp covers the pair.
    Causal mask added with accumulating identity x (-1e9 triangle)
    matmuls on diagonal blocks; fully-masked blocks skipped, diagonal
    blocks narrowed to the unmasked column range.
  - ctxT[d, q] (+ sums row) accumulated per head over k tiles in PSUM.
  - Normalize straight out of PSUM: linv = 1/sums (DVE reciprocal),
    broadcast across partitions on GPSIMD (partition_broadcast - no DRAM
    bounce), multiply on DVE -> ctxn (bf16).
  - outT partial [e, q] = Wo_shard^T @ ctxn; PSUM -> SBUF bf16 copy on
    GPSIMD, DMA'd out as bf16.
  - Emission is chunk-pipelined: attention of chunk qc interleaves
    "filler" PE work (projections of chunk qc+1, out-projection of chunk
    qc-1) into the gaps where PE would otherwise wait on ScalarE's exp,
    so no engine-idle projection phase exists.

Host side: gather 8 partial bf16 outT tensors, out[b] = sum_g partial^T
(fp32) + bo.
"""

import numpy as np
import ml_dtypes

import bass_rust
import concourse.bass as bass
import concourse.tile as tile
from concourse import mybir
from concourse.tile import TileContext
from concourse.bass_utils import run_bass_kernel_spmd
from concourse.vector_clock import ScopedClock

# ---------------------------------------------------------------------------
# Workaround: this environment's walrus build caps the number of sync-wait
# commands encodable on a single instruction. Redistribute excess waits onto
# single-wait NOPs inserted immediately before the instruction on the same
# engine queue (program order preserves the blocking semantics).
# ---------------------------------------------------------------------------


def _patched_drain_and_barrier(self, tick_clock, wait_clock):
    nop0 = self.nc.sync.nop(nofuse=True)
    wait_clock.add_sem_waits(nop0.ins, ScopedClock({None: tick_clock.global_clock}))
    si = nop0.ins.sync_info
    if si is not None and si.on_wait and len(si.on_wait) > 1:
        waits = list(si.on_wait)
        nop0.ins.sync_info = bass_rust.SyncInfo(
            on_wait=[waits[0]], on_update=list(si.on_update or [])
        )
        for w in waits[1:]:
            n = self.nc.sync.nop(nofuse=True)
            n.ins.sync_info = bass_rust.SyncInfo(on_wait=[w], on_update=[])
    self.nc.sync.drain()
    self.nc.all_engine_barrier()
    assert self.sems is not None
    popped = self.nc._tile_sem_poison_stack.pop()
    assert popped is self._sem_poison
    self.nc.clear_and_free_semaphores(list(self.sems.allocated().values()))
    self.nc.all_engine_barrier()


tile.TileContext._drain_and_barrier = _patched_drain_and_barrier

MAX_WAITS = 1


def split_excess_waits(nc, max_waits=None):
    if max_waits is None:
        max_waits = MAX_WAITS
    for f in nc.m.functions:
        for bb in f.blocks:
            insts = bb.instructions
            out = []
            changed = False
            for inst in insts:
                si = inst.sync_info
                waits = list(si.on_wait) if si is not None and si.on_wait else []
                if len(waits) > max_waits:
                    changed = True
                    excess, keep = waits[:-max_waits], waits[-max_waits:]
                    for w in excess:
                        out.append(mybir.InstNoOp(
                            name=nc.get_next_instruction_name(),
                            engine=inst.engine,
                            bass_nofuse=True,
                            sync_info=mybir.SyncInfo(on_wait=[w], on_update=[]),
                        ))
                    inst.sync_info = mybir.SyncInfo(
                        on_wait=keep, on_update=list(si.on_update or [])
                    )
                out.append(inst)
            if changed:
                bb.instructions = out


# ---------------------------------------------------------------------------
# Problem constants (hardcoded per the harness contract).
# ---------------------------------------------------------------------------

B, S, E = 2, 2048, 1024
H, D = 16, 64
NCORES, TP = 8, 4
HPC = H // TP          # heads per core = 4
DH = HPC * D           # head-dim columns per core = 256
P = 128                # partitions
QC = 512               # q chunk (free dim of score matmuls)
NQC = S // QC          # 4 q chunks
NKT = S // P           # 16 k tiles
ET = E // P            # 8 e tiles
NM = QC // P           # 4 k-tiles per q chunk
SCALE = 1.0 / np.sqrt(np.float32(D))
NEG = -1.0e9
# fp8 Q/K weights are prescaled so N(0, 0.02) lands in e4m3's normal range
# (min normal ~0.0156); folded back out of the scores via the exp scale
QK_W_SCALE = 32.0

OPTS = {
    "st_bufs": 2,      # [P, 2, QC] fp32 = 2 PSUM banks each
    "pj_bufs": 2,      # [P, QC] fp32 = 1 bank each (proj + outproj filler)
    "stx_bufs": 8,
    "fp8_qk": True,    # fp8 DoubleRow Q/K projections
    "warmup_mms": 40,  # PE p-state warmup matmuls during initial DMA wait
    "sel_mask": True,  # causal mask via GPSIMD affine_select on exp output
                       # (replaces 2 PE matmuls + 2 ldweights per diag tile)
}

BF = mybir.dt.bfloat16
F32 = mybir.dt.float32
F8 = mybir.dt.float8e4
Exp = mybir.ActivationFunctionType.Exp
MULT = mybir.AluOpType.mult
DR = mybir.MatmulPerfMode.DoubleRow


class _Persist:
    pass


def _phase_view(ps, phase):
    """Shallow view of the persistent set with the phase's input tiles."""
    import copy
    v = copy.copy(ps)
    b = ps.in_bufs[phase % len(ps.in_bufs)]
    for a in ("xt_t", "xt8", "wq_sb", "wk_sb", "wv_sb", "wo_sb"):
        if hasattr(b, a):
            setattr(v, a, getattr(b, a))
    return v


def _emit_dmas(nc, io, ps):
    if OPTS.get("compute_only"):
        # keep only the output DMAs' sources defined; skip big input loads
        return
    nc.sync.dma_start(ps.wv_sb[:], io.wv)
    if OPTS["fp8_qk"]:
        nc.sync.dma_start(ps.wq_sb[:], io.wq)
        nc.sync.dma_start(ps.wk_sb[:], io.wk)
        nc.sync.dma_start(ps.xt8[:, :, 0:QC], io.xt8[:, :, 0:QC])
    for et in range(ET):
        nc.sync.dma_start(ps.xt_t[et][:, 0:QC], io.xt[:, et, 0:QC])
    if not OPTS["fp8_qk"]:
        nc.sync.dma_start(ps.wq_sb[:], io.wq)
        nc.sync.dma_start(ps.wk_sb[:], io.wk)
    nc.sync.dma_start(ps.wo_sb[:], io.wo)
    if OPTS["fp8_qk"]:
        nc.sync.dma_start(ps.xt8[:, :, QC:], io.xt8[:, :, QC:])
    for et in range(ET):
        nc.sync.dma_start(ps.xt_t[et][:, QC:], io.xt[:, et, QC:])


# optional emission-label hook for trace attribution (set by gaps.py)
_label_hook = None


def _lbl(tag):
    if _label_hook is not None:
        _label_hook(tag)


def _declare_io(nc):
    io = _Persist()
    # inputs pre-arranged on host to [partition, plane, free] so every DMA
    # is contiguous per partition line
    io.xt = nc.dram_tensor("xt", [P, ET, S], BF, kind="ExternalInput").ap()
    io.wv = nc.dram_tensor("wv", [P, ET, DH], BF, kind="ExternalInput").ap()
    io.wo = nc.dram_tensor("wo", [P, DH // P, E], BF, kind="ExternalInput").ap()
    if OPTS["fp8_qk"]:
        io.xt8 = nc.dram_tensor("xt8", [P, ET, S], F8, kind="ExternalInput").ap()
        io.wq = nc.dram_tensor("wq", [P, ET, DH], F8, kind="ExternalInput").ap()
        io.wk = nc.dram_tensor("wk", [P, ET, DH], F8, kind="ExternalInput").ap()
    else:
        io.wq = nc.dram_tensor("wq", [P, ET, DH], BF, kind="ExternalInput").ap()
        io.wk = nc.dram_tensor("wk", [P, ET, DH], BF, kind="ExternalInput").ap()
    io.outp = nc.dram_tensor("outp", [E, S], BF, kind="ExternalOutput").ap()
    return io


def _persistent(ctx, tc):
    nc = tc.nc
    ps = _Persist()
    consts = ctx.enter_context(tc.tile_pool(name="consts", bufs=1))

    qk_dt = F8 if OPTS["fp8_qk"] else BF
    # Input tiles are ping-pong buffered across bench iterations so
    # iteration i+1's DMAs never wait on iteration i's reads (the graded
    # single-iteration path only ever touches phase 0).
    nph = 2 if OPTS.get("pingpong", True) else 1
    ps.in_bufs = []
    for ph in range(nph):
        b = _Persist()
        # xt split per e-tile so V projections start as each chunk lands
        b.xt_t = [consts.tile([P, S], BF, tag=f"xt{ph}_{et}",
                              name=f"xt{ph}_{et}") for et in range(ET)]
        if OPTS["fp8_qk"]:
            b.xt8 = consts.tile([P, ET, S], F8, tag=f"xt8_{ph}")
        b.wq_sb = consts.tile([P, ET, DH], qk_dt, tag=f"wq{ph}")
        b.wk_sb = consts.tile([P, ET, DH], qk_dt, tag=f"wk{ph}")
        b.wv_sb = consts.tile([P, ET, DH], BF, tag=f"wv{ph}")
        b.wo_sb = consts.tile([P, DH // P, E], BF, tag=f"wo{ph}")
        ps.in_bufs.append(b)
    ps.qt_q = [consts.tile([P, DH // P, QC], BF, tag=f"qt{qc}",
                           name=f"qt{qc}") for qc in range(NQC)]
    ps.kt_q = [consts.tile([P, DH // P, QC], BF, tag=f"kt{qc}",
                           name=f"kt{qc}") for qc in range(NQC)]
    ps.v_t = [consts.tile([P, HPC, 66], BF, tag=f"v{ki}", name=f"v{ki}")
              for ki in range(NKT)]
    ps.ctxn_q = [consts.tile([P, DH // P, QC], BF, tag=f"ctxn{qc}",
                             name=f"ctxn{qc}") for qc in range(NQC)]
    ps.ident = consts.tile([P, P], BF, tag="ident")
    ps.masks = consts.tile([P, NM, QC], BF, tag="masks")
    ps.ones64 = consts.tile([1, 64], BF, tag="ones64")
    nc.gpsimd.memset(ps.ones64[:], 1.0)
    # warmup-matmul operand: DVE memset is ready ~2us before the gpsimd
    # identity init, so the p-state warmup starts at t~0
    ps.warm = consts.tile([P, P], BF, tag="warm")
    nc.vector.memset(ps.warm[:], 0.0)

    # identity (for the mask-add matmul)
    nc.gpsimd.memset(ps.ident[:], 0.0)
    nc.gpsimd.affine_select(
        out=ps.ident[:], in_=ps.ident[:],
        compare_op=mybir.AluOpType.not_equal, fill=1.0,
        base=0, pattern=[[-1, P]], channel_multiplier=1,
    )
    # mask[m][p, fq] = 0 where fq >= p + 128*m else NEG
    for m in range(NM):
        nc.gpsimd.memset(ps.masks[:, m, :], 0.0)
        nc.gpsimd.affine_select(
            out=ps.masks[:, m, :], in_=ps.masks[:, m, :],
            compare_op=mybir.AluOpType.is_ge, fill=NEG,
            base=-P * m, pattern=[[1, QC]], channel_multiplier=-1,
        )
    # ones column for V_aug
    for ki in range(NKT):
        nc.gpsimd.memset(ps.v_t[ki][:, :, 64:66], 0.0)
        nc.gpsimd.memset(ps.v_t[ki][:, :, 64:65], 1.0)

    ps.stx_pool = ctx.enter_context(
        tc.tile_pool(name="stx", bufs=OPTS["stx_bufs"]))
    ps.linv_pool = ctx.enter_context(tc.tile_pool(name="linv", bufs=2))
    ps.ctxu_pool = ctx.enter_context(tc.tile_pool(name="ctxu", bufs=2))
    ps.ob_pool = ctx.enter_context(tc.tile_pool(name="ob", bufs=4))
    return ps


def _emit_qk_proj(nc, ps, pjp, qc, w_sb, dst_q):
    """One filler piece per (weight, dt): QT/KT[:, dt, qc-chunk]."""
    qs = slice(qc * QC, (qc + 1) * QC)
    for dt in range(DH // P):
        _lbl(f"qkproj q{qc} dt{dt}")
        psum = pjp.tile([P, QC], F32, tag="pj", name="pj")
        if OPTS["fp8_qk"]:
            for ep in range(ET // 2):
                nc.tensor.matmul(
                    psum[:],
                    lhsT=w_sb[:, 2 * ep:2 * ep + 2, dt * P:(dt + 1) * P],
                    rhs=ps.xt8[:, 2 * ep:2 * ep + 2, qs],
                    start=(ep == 0), stop=(ep == ET // 2 - 1),
                    perf_mode=DR,
                )
        else:
            for et in range(ET):
                nc.tensor.matmul(
                    psum[:],
                    lhsT=w_sb[:, et, dt * P:(dt + 1) * P],
                    rhs=ps.xt_t[et][:, qs],
                    start=(et == 0), stop=(et == ET - 1),
                )
        nc.vector.tensor_copy(out=dst_q[qc][:, dt, :], in_=psum[:])


def _emit_v_proj(nc, ps, pjp, st):
    """One filler piece per 128-row k-tile of V."""
    _lbl(f"vproj st{st}")
    psum = pjp.tile([P, QC], F32, tag="pj", name="pjv")
    for et in range(ET):
        nc.tensor.matmul(
            psum[:, 0:DH],
            lhsT=ps.xt_t[et][:, st * P:(st + 1) * P],
            rhs=ps.wv_sb[:, et, :],
            start=(et == 0), stop=(et == ET - 1),
        )
    nc.vector.tensor_copy(
        out=ps.v_t[st][:, :, 0:64],
        in_=psum[:, 0:DH].rearrange("p (h d) -> p h d", h=HPC),
    )


def _emit_outproj(nc, io, ps, pjp, qc, et, on_act=False, wide_pool=None):
    """One filler piece per e-tile of the output projection of chunk qc.

    on_act: do the PSUM->SBUF copy on the Act engine (Copy activation) -
    used in the drain tail where ScalarE is idle but DVE is the
    bottleneck of the outproj pipeline.
    wide_pool: borrow a [P, 2, QC] pool (the attention st pool, idle in
    the tail) for extra PSUM slots so the drain pipelines deeper."""
    _lbl(f"outproj q{qc} et{et}")
    if wide_pool is not None:
        psum = wide_pool.tile([P, 2, QC], F32, tag="st", name="st")[:, 0, :]
    else:
        psum = pjp.tile([P, QC], F32, tag="pj", name="po")
    for cc in range(DH // P):
        nc.tensor.matmul(
            psum[:],
            lhsT=ps.wo_sb[:, cc, et * P:(et + 1) * P],
            rhs=ps.ctxn_q[qc][:, cc, :],
            start=(cc == 0), stop=(cc == DH // P - 1),
        )
    ob = ps.ob_pool.tile([P, QC], BF, tag="ob", name="ob")
    if on_act:
        nc.scalar.copy(out=ob[:], in_=psum[:])
    else:
        nc.vector.tensor_copy(out=ob[:], in_=psum[:])
    nc.sync.dma_start(
        io.outp.rearrange("(eo p) q -> p eo q", p=P)[
            :, et, qc * QC:(qc + 1) * QC
        ],
        ob[:],
    )


def proj_fillers(nc, ps, pjp, qc):
    fs = []
    for w_sb, dst in ((ps.wq_sb, ps.qt_q), (ps.wk_sb, ps.kt_q)):
        fs.append(lambda w=w_sb, d=dst, q=qc: _emit_qk_proj(nc, ps, pjp, q, w, d))
    for st in range(qc * NM, (qc + 1) * NM):
        fs.append(lambda s=st: _emit_v_proj(nc, ps, pjp, s))
    return fs


def outproj_fillers(nc, io, ps, pjp, qc, alt_act=False):
    # alt_act: alternate copies between DVE and Act so neither serializes
    # the drain
    return [lambda e=et: _emit_outproj(nc, io, ps, pjp, qc, e,
                                       on_act=alt_act and e % 2 == 0)
            for et in range(ET)]


def _normalize_recip(nc, ps, pvs, qc, cc):
    """Stage 1 of pair normalization (DVE only): evacuate the pair's PV
    accumulators from PSUM to SBUF (frees the pv banks for the next
    pair as early as possible) and compute linv = 1/sums."""
    _lbl(f"recip q{qc} cc{cc}")
    ctxu = ps.ctxu_pool.tile([65, 2, QC], F32, tag="ctxu", name="ctxu")
    nc.vector.tensor_copy(out=ctxu[:], in_=pvs[0:65, :, :])
    linv = ps.linv_pool.tile([1, 2, QC], BF, tag="linv", name="linv")
    with nc.allow_low_precision(reason="bf16 linv for broadcast matmul"):
        nc.vector.reciprocal(out=linv[:], in_=ctxu[64:65, :, :])
    return ctxu, linv


def _normalize_pair(nc, ps, pjp, norm, qc, cc):
    """Stage 2 (off the pv-bank critical path): broadcast linv across
    partitions via K=1 outer-product matmuls (ones x linv -> PSUM) and
    multiply the SBUF copy by them on DVE -> ctxn (bf16)."""
    ctxu, linv = norm
    _lbl(f"normalize q{qc} cc{cc}")
    for i in range(2):
        lb = pjp.tile([P, QC], F32, tag="pj", name="linvb")
        nc.tensor.matmul(
            lb[0:64, :],
            lhsT=ps.ones64[:],
            rhs=linv[0:1, i, :],
            start=True, stop=True,
        )
        nc.vector.tensor_tensor(
            ps.ctxn_q[qc][64 * i:64 * i + 64, cc, :],
            ctxu[0:64, i, :], lb[0:64, :], MULT,
        )


def _attention_chunk(tc, ps, stp, pvp, pjp, qc, fillers, pending):
    """Attention for chunk qc; pops filler closures into PE gaps.

    `pending` holds the not-yet-emitted normalize of the previous head
    pair: it is emitted between the first exp and the first PV of the
    next pair, so its PE part (the broadcast matmul) never blocks the
    in-order PE queue behind a long DVE chain, and the single pv-pair
    PSUM buffer is released just in time.
    """
    nc = tc.nc
    nk = (qc + 1) * NM
    # spread filler consumption evenly over this chunk's (2*nk - 2)
    # filler-eligible iterations: [deficit accumulator, per-iter rate]
    reserve = 5 if qc == NQC - 1 else 0
    budget = [1.0, max(0, len(fillers) - reserve) / max(1, 2 * nk - 2)]
    for hp in range(HPC // 2):
        cc = hp
        pvs = pvp.tile([65, 2, QC], F32, tag="pv", name="pv")
        pv_q = []  # software pipeline: PV(ki) emitted one iteration late

        def emit_pv():
            ki_, off_, stx_ = pv_q.pop(0)
            _lbl(f"PV q{qc} hp{hp} ki{ki_}")
            for i in range(2):
                h = 2 * hp + i
                nc.tensor.matmul(
                    pvs[0:65, i, off_:],
                    lhsT=ps.v_t[ki_][:, h, 0:65],
                    rhs=stx_[:, i, off_:],
                    start=(ki_ == 0), stop=(ki_ == nk - 1),
                )

        for ki in range(nk):
            diag = ki >= qc * NM
            m = ki - qc * NM if diag else 0
            off = P * m if diag else 0
            _lbl(f"ST q{qc} hp{hp} ki{ki}")
            st_ps = stp.tile([P, 2, QC], F32, tag="st", name="st")
            kqc, kf = ki // NM, (ki % NM) * P
            for i in range(2):
                pr = 64 * i
                nc.tensor.matmul(
                    st_ps[:, i, off:],
                    lhsT=ps.kt_q[kqc][pr:pr + 64, cc, kf:kf + P],
                    rhs=ps.qt_q[qc][pr:pr + 64, cc, off:],
                    start=True,
                    stop=(not diag or bool(OPTS.get("no_mask"))
                          or bool(OPTS.get("sel_mask"))),
                )
            if diag and not OPTS.get("no_mask") and not OPTS.get("sel_mask"):
                for i in range(2):
                    nc.tensor.matmul(
                        st_ps[:, i, off:off + P],
                        lhsT=ps.ident[:],
                        rhs=ps.masks[:, m, off:off + P],
                        start=False, stop=True,
                    )
            budget[0] += budget[1]
            # front-load the pair-start transient (ki 1-2): Act is still
            # draining the previous pair's exps there, so PE has extra
            # slack for filler work
            cap = 2.0 if ki in (1, 2) else 1.0
            ndone = 0.0
            # skip the pair's last iteration: a filler's DVE copy queued
            # there would delay the critical ctxu evacuation that frees
            # the pv banks for the next pair
            while (ki >= 1 and ki < nk - 1 and fillers
                   and budget[0] >= 1.0 and ndone < cap):
                fillers.popleft()()
                budget[0] -= 1.0
                ndone += 1.0
            _lbl(f"exp q{qc} hp{hp} ki{ki}")
            stx = ps.stx_pool.tile([P, 2, QC], BF, tag="stx", name="stx")
            escale = SCALE / (QK_W_SCALE ** 2) if OPTS["fp8_qk"] else SCALE
            if OPTS.get("exp_split"):
                for i in range(2):
                    nc.scalar.activation(
                        out=stx[:, i, off:], in_=st_ps[:, i, off:], func=Exp,
                        scale=float(escale),
                    )
            else:
                nc.scalar.activation(
                    out=stx[:, :, off:], in_=st_ps[:, :, off:], func=Exp,
                    scale=float(escale),
                )
            if diag and OPTS.get("sel_mask"):
                # zero the above-diagonal triangle of exp'd scores for both
                # heads in one GPSIMD op (predicate ignores the head plane):
                # keep where fq >= p, else 0
                nc.gpsimd.affine_select(
                    out=stx[:, :, off:off + P], in_=stx[:, :, off:off + P],
                    compare_op=mybir.AluOpType.is_ge, fill=0.0,
                    base=0, pattern=[[0, 2], [1, P]], channel_multiplier=-1,
                )
            if ki == 2 and pending[0] is not None:
                pending[0]()
                pending[0] = None
            pv_q.append((ki, off, stx))
            if len(pv_q) > 4:
                emit_pv()
        while pv_q:
            emit_pv()
        norm = _normalize_recip(nc, ps, pvs, qc, cc)
        pending[0] = (lambda n=norm, q=qc, c=cc:
                      _normalize_pair(nc, ps, pjp, n, q, c))


def _iteration(tc, io, ps, first=True):
    from collections import deque
    nc = tc.nc

    if OPTS.get("dma_only"):
        _emit_dmas(nc, io, ps)
        return

    # DMAs ordered so chunk-0 dependencies land first
    _emit_dmas(nc, io, ps)
    if OPTS.get("compute_only"):
        return

    with tc.tile_pool(name="st", bufs=OPTS["st_bufs"], space="PSUM") as stp, \
         tc.tile_pool(name="pv", bufs=1, space="PSUM") as pvp, \
         tc.tile_pool(name="pj", bufs=OPTS["pj_bufs"], space="PSUM") as pjp:
        # PE p-state warmup during the initial DMA wait: dependency-free
        # matmuls on the persistent identity keep the clock ramp hot so
        # the first projections run at full speed
        _lbl("warmup")
        # p-state warmup only pays off when PE has been idle (the first
        # iteration); later chained iterations arrive already ramped
        for w in range(OPTS["warmup_mms"] if first else 0):
            wt = pjp.tile([P, QC], F32, tag="pj", name="warm")
            nc.tensor.matmul(wt[:, 0:P], lhsT=ps.warm[:], rhs=ps.warm[:],
                             start=True, stop=True)
        # prologue: chunk-0 projections emitted directly
        for f in proj_fillers(nc, ps, pjp, 0):
            f()
        fillers = deque()
        pending = [None]
        for qc in range(NQC):
            # proj(qc) must be fully emitted before attention(qc) reads it
            for _ in range(2):
                if fillers:
                    fillers.popleft()()
            if pending[0] is not None:
                pending[0]()
                pending[0] = None
            while fillers:
                fillers.popleft()()
            if qc + 1 < NQC:
                fillers.extend(proj_fillers(nc, ps, pjp, qc + 1))
            # out-projections are deferred TWO chunks so the later (longer,
            # Act-bound) chunks get more PE filler work: outproj(0)->chunk 2,
            # outproj(1) and outproj(2)->chunk 3
            if qc >= 2:
                fillers.extend(outproj_fillers(nc, io, ps, pjp, qc - 2))
            if qc == NQC - 1:
                fillers.extend(outproj_fillers(nc, io, ps, pjp, qc - 1))
            _attention_chunk(tc, ps, stp, pvp, pjp, qc, fillers, pending)
        for _ in range(3):
            if fillers:
                fillers.popleft()()
        if pending[0] is not None:
            pending[0]()
            pending[0] = None
        while fillers:
            fillers.popleft()()
        for et in range(ET):
            _emit_outproj(nc, io, ps, pjp, NQC - 1, et, on_act=et % 2 == 0,
                          wide_pool=stp if et % 2 == 0 else None)


_NC_CACHE = {}


def build_nc(iters=1):
    if iters not in _NC_CACHE:
        from contextlib import ExitStack
        nc = bass.Bass("TRN2", target_bir_lowering=False, debug=False)
        with TileContext(nc) as tc, ExitStack() as es:
            io = _declare_io(nc)
            ps = _persistent(es, tc)
            for it in range(iters):
                _iteration(tc, io, _phase_view(ps, it), first=(it == 0))
        split_excess_waits(nc)
        _NC_CACHE[iters] = nc
    return _NC_CACHE[iters]


def make_in_maps(embeddings, wq, wk, wv, wo):
    bf = ml_dtypes.bfloat16
    f8 = ml_dtypes.float8_e4m3
    in_maps = []
    for c in range(NCORES):
        b, g = c // TP, c % TP
        cols = slice(g * DH, (g + 1) * DH)

        def _arr(a, dt):  # [(c p), f] -> [p, c, f] contiguous
            n = a.shape[0] // P
            return np.ascontiguousarray(
                a.reshape(n, P, a.shape[1]).transpose(1, 0, 2)).astype(dt)

        m = {
            "xt": _arr(embeddings[b].T, bf),
            "wv": _arr(wv[:, cols], bf),
            "wo": _arr(wo[cols, :], bf),
        }
        if OPTS["fp8_qk"]:
            m["xt8"] = _arr(embeddings[b].T, f8)
            m["wq"] = _arr(wq[:, cols] * QK_W_SCALE, f8)
            m["wk"] = _arr(wk[:, cols] * QK_W_SCALE, f8)
        else:
            m["wq"] = _arr(wq[:, cols], bf)
            m["wk"] = _arr(wk[:, cols], bf)
        in_maps.append(m)
    return in_maps


def assemble(results, bo):
    out = np.zeros((B, S, E), dtype=np.float32)
    for c in range(NCORES):
        b = c // TP
        out[b] += results[c]["outp"].T.astype(np.float32)
    out += bo.astype(np.float32)
    return out


def kernel(embeddings, wq, wk, wv, wo, bo):
    embeddings = np.asarray(embeddings)
    nc = build_nc()
    in_maps = make_in_maps(embeddings, np.asarray(wq), np.asarray(wk),
                           np.asarray(wv), np.asarray(wo))
    res = run_bass_kernel_spmd(nc, in_maps, core_ids=list(range(NCORES)),
                               trace=False)
    return assemble(res.results, np.asarray(bo))

